# revision 20
# baseline (speedup 1.0000x reference)
"""GRUCell4RNMT fused Trainium2 kernel, data-parallel over 8 NeuronCores.

Reference computation (per batch row b):
    comb  = concat([x, h]) @ Wt.T            # [B, 2048]
    mu, var = moments over all 2048 comb features (joint LayerNorm)
    normed  = (comb - mu) * rsqrt(var+eps) * ln_w + ln_b
    ig, fg  = sigmoid(normed).split(2)
    hidden  = tanh(x @ Wi.T + bi + ig * (h @ Wh.T + bh))
    out     = (1 - fg) * hidden + fg * h

Strategy: shard batch 16384 -> 8 x 2048; 16 batch tiles of 128 rows per
core. Mixed-precision matmuls tuned to the error budget (rel < 2e-2):
  - comb ig half (cols 0:1024): fp8-e4m3 DoubleRow (2 k-tiles per
    instruction -> 2.1x bf16 FLOP rate). Gate errors are damped by the
    sigmoid slope and the tanh, so 1-term fp8 passes (sim: 1.5e-2).
  - comb fg half + x@Wi.T: bf16 (the fg*(state-hidden) path amplifies
    errors by |s-h| up to ~5, fp8 fails there).
  - h@Wh.T: fp8 DoubleRow (damped by ig and tanh).
LayerNorm is scale-invariant, so comb runs at 64x scale (fp8 weights
need x64 to escape e4m3 subnormals; the bf16 fg weights carry the same
x64 exactly). hWh's 1/64 dequant folds into its bias-add stt.

Scheduling (measured on this hw): DR matmuls stream 1 out/cycle
(PSUM-write-bound) and DR LDWEIGHTS does not hide behind DR matmuls,
so the DR pairs are interleaved among the bf16 fg k-tiles (ldw
prefetches under bf16 streaming); hwh shares the pair-4..7 stationaries
with ig; xWi runs last so its ~2.9us covers the comb eviction. comb is
evicted from PSUM to SBUF bf16 by 4 ACT copies right after the comb
matmuls (frees the 4 pc banks before the next tile needs them), then
bn_stats/bn_aggr on DVE, sigmoid fused on ACT (scale=rstd,
bias=-mu*rstd per partition), tanh on ACT, gate/mix chain on DVE in
bf16 where safe. ln_w==1/ln_b==0 (as produced by setup_inputs) enables
the fused-sigmoid path; a general program is built on demand otherwise.
Steady state: PE ~12.9us/tile busy, DVE ~8us, ACT ~6us -> ~212us/pass
vs a ~207us PE floor (bf16 512-wide mm = 182ns, fp8-DR = 174ns).
"""

import numpy as np
import ml_dtypes

from concourse import bass, mybir, tile
from concourse.bass_utils import run_bass_kernel_spmd
from concourse.vector_clock import ScopedClock

BF16 = ml_dtypes.bfloat16
E4M3 = ml_dtypes.float8_e4m3
F32 = mybir.dt.float32
BF = mybir.dt.bfloat16
FP8 = mybir.dt.float8e4
AF = mybir.ActivationFunctionType
ALU = mybir.AluOpType
DRMODE = mybir.MatmulPerfMode.DoubleRow

N_CORES = 8
B = 16384
O = 1024
BL = B // N_CORES          # 2048 rows per core
N_BT = BL // 128           # 16 batch tiles per core
N_K = 16                   # contraction tiles (8 from x, 8 from h)
LN_EPS = 1e-6
WSCALE = 64.0              # comb & hwh computed at 64x scale


class _TC(tile.TileContext):
    """TileContext whose kernel-tail drain honors the 1-wait-per-
    instruction ISA cap: extra waits move onto dedicated drains."""

    def _drain_and_barrier(self, tick_clock, wait_clock):
        drain_inst = self.nc.sync.drain()
        wait_clock.add_sem_waits(
            drain_inst.ins, ScopedClock({None: tick_clock.global_clock})
        )
        si = drain_inst.ins.sync_info
        if si is not None and si.on_wait and len(si.on_wait) > 1:
            waits = list(si.on_wait)
            SI = type(si)
            si.on_wait = [waits[0]]
            for w in waits[1:]:
                extra = self.nc.sync.drain()
                extra.ins.sync_info = SI(on_wait=[w], on_update=[])
        self.nc.all_engine_barrier()
        assert self.sems is not None
        popped = self.nc._tile_sem_poison_stack.pop()
        assert popped is self._sem_poison
        self.nc.clear_and_free_semaphores(list(self.sems.allocated().values()))
        self.nc.all_engine_barrier()


def _split_multi_waits(nc):
    """This walrus build accepts 1 sync wait per instruction (2 on
    EventSemaphore). Tile's scheduler can emit more; move the extras
    onto EventSemaphore carriers inserted just before the offender on
    the same engine (identical blocking semantics)."""
    for fn in nc.m.functions:
        for blk in fn.blocks:
            il = blk.instructions
            i = 0
            while i < len(il):
                inst = il[i]
                si = inst.sync_info
                cap = 2 if isinstance(inst, mybir.InstEventSemaphore) else 1
                if si is not None and si.on_wait and len(si.on_wait) > cap:
                    waits = list(si.on_wait)
                    SI = type(si)
                    si.on_wait = waits[:cap]
                    extra = waits[cap:]
                    pos = i
                    while extra:
                        chunk, extra = extra[:2], extra[2:]
                        ev = mybir.InstEventSemaphore(
                            name=nc.get_next_instruction_name(), ins=[], outs=[]
                        )
                        ev.engine = inst.engine
                        ev.sync_info = SI(on_wait=chunk, on_update=[])
                        nc.register_instruction(ev, overwrite=True)
                        il.insert(pos, ev)
                        pos += 1
                        i += 1
                i += 1


def build_program(n_bt=N_BT, reps=1, identity_ln=True, trace_sim=False,
                  mm_only=False):
    nc = bass.Bass()
    bl = n_bt * 128

    # fp8 [x;h]^T and bf16 [x;h]^T, k-major
    x8 = nc.declare_dram_parameter("x8", [2048, bl], FP8, isOutput=False)
    xb = nc.declare_dram_parameter("xb", [2048, bl], BF, isOutput=False)
    st = nc.declare_dram_parameter("st", [bl, O], BF, isOutput=False)
    # weights (prepacked on host; see _prep_inputs)
    wig = nc.declare_dram_parameter("wig", [1024, 2, 1024], FP8, isOutput=False)
    wfg = nc.declare_dram_parameter("wfg", [2048, 1024], BF, isOutput=False)
    wi = nc.declare_dram_parameter("wi", [1024, 1024], BF, isOutput=False)
    wh = nc.declare_dram_parameter("wh", [512, 2, 1024], FP8, isOutput=False)
    bib = nc.declare_dram_parameter("bib", [128, O], BF, isOutput=False)
    bhb = nc.declare_dram_parameter("bhb", [128, O], BF, isOutput=False)
    if not identity_ln:
        lnw = nc.declare_dram_parameter("lnw", [128, 2048], BF, isOutput=False)
        lnb = nc.declare_dram_parameter("lnb", [128, 2048], BF, isOutput=False)
    out = nc.declare_dram_parameter("out", [bl, O], F32, isOutput=True)

    x8_r = x8[:].rearrange("(k p) n -> p k n", p=128)
    xb_r = xb[:].rearrange("(k p) n -> p k n", p=128)

    with _TC(nc, trace_sim=trace_sim) as tc:
        with (
            tc.tile_pool(name="wp", bufs=1) as wp,
            tc.tile_pool(name="cp", bufs=1) as cp,
            tc.tile_pool(name="xp", bufs=3) as xp,
            tc.tile_pool(name="sp", bufs=3) as sp,
            tc.tile_pool(name="ep", bufs=3) as ep,
            tc.tile_pool(name="pc_p", bufs=1, space="PSUM") as pc_p,
            tc.tile_pool(name="pa_p", bufs=1, space="PSUM") as pa_p,
            tc.tile_pool(name="pb_p", bufs=1, space="PSUM") as pb_p,
        ):
            wig_t = [wp.tile([128, 2, 1024], FP8, tag=f"wig{p}",
                             name=f"wig{p}") for p in range(8)]
            wfg_t = [wp.tile([128, 1024], BF, tag=f"wfg{k}",
                             name=f"wfg{k}") for k in range(16)]
            wi_t = [wp.tile([128, 1024], BF, tag=f"wi{k}",
                            name=f"wi{k}") for k in range(8)]
            wh_t = [wp.tile([128, 2, 1024], FP8, tag=f"wh{p}",
                            name=f"wh{p}") for p in range(4)]
            # weight DMAs in consumption order, issued lazily 2 ahead
            # of first use so the PE starts early.
            wloads = [(wig_t[p], wig[p * 128:(p + 1) * 128]) for p in range(4)]
            for p in range(4, 8):
                wloads.append((wig_t[p], wig[p * 128:(p + 1) * 128]))
                wloads.append((wh_t[p - 4], wh[(p - 4) * 128:(p - 3) * 128]))
            wloads += [(wfg_t[k], wfg[k * 128:(k + 1) * 128])
                       for k in range(16)]
            wloads += [(wi_t[k], wi[k * 128:(k + 1) * 128]) for k in range(8)]
            n_loaded = [0]

            def load_w_until(i):
                while n_loaded[0] < min(i, len(wloads)):
                    t, src = wloads[n_loaded[0]]
                    nc.sync.dma_start(t[:], src[:])
                    n_loaded[0] += 1

            load_w_until(2)

            lnw_t = lnb_t = None
            if not identity_ln:
                lnw_t = cp.tile([128, 2048], BF, tag="lnw")
                lnb_t = cp.tile([128, 2048], BF, tag="lnb")
            bib_t = cp.tile([128, O], BF, tag="bib")
            bhb_t = cp.tile([128, O], BF, tag="bhb")
            eps_t = cp.tile([128, 1], F32, tag="eps")

            for bt_r in range(n_bt * reps):
                bt = bt_r % n_bt
                first = bt_r == 0
                x8_t = xp.tile([128, N_K, 128], FP8, tag="x8")
                nc.sync.dma_start(x8_t[:], x8_r[:, :, bt * 128:(bt + 1) * 128])
                xb_t = xp.tile([128, N_K, 128], BF, tag="xb")
                nc.sync.dma_start(xb_t[:], xb_r[:, :, bt * 128:(bt + 1) * 128])
                st_t = ep.tile([128, O], BF, tag="st")
                nc.sync.dma_start(st_t[:], st[bt * 128:(bt + 1) * 128, :])

                pc = pc_p.tile([128, 2048], F32, tag="pc")
                pa = pa_p.tile([128, O], F32, tag="pa")
                pb = pb_p.tile([128, O], F32, tag="pb")

                # --- comb phase: interleave bf16 fg k-tiles with the
                # DR ig pairs so DR ldweights prefetch under bf16
                # streaming; pairs 4-7 also feed hwh under the same
                # stationary (saves 4 ldweights) ---
                for p in range(8):
                    if first:
                        load_w_until((p if p < 4 else 2 * p - 3) + 3)
                        load_w_until(12 + 2 * p + 2 + 3)
                    for k in (2 * p, 2 * p + 1):
                        lhs = xb_t[:, k, :]
                        for c in range(2):
                            nc.tensor.matmul(
                                pc[:, 1024 + c * 512:1024 + (c + 1) * 512],
                                lhs,
                                wfg_t[k][:, c * 512:(c + 1) * 512],
                                start=(k == 0), stop=(k == 15),
                            )
                    lhs = x8_t[:, 2 * p:2 * p + 2, :]
                    for c in range(2):
                        nc.tensor.matmul(
                            pc[:, c * 512:(c + 1) * 512],
                            lhs,
                            wig_t[p][:, :, c * 512:(c + 1) * 512],
                            start=(p == 0), stop=(p == 7),
                            perf_mode=DRMODE,
                        )
                    if p >= 4:
                        for c in range(2):
                            nc.tensor.matmul(
                                pb[:, c * 512:(c + 1) * 512],
                                lhs,
                                wh_t[p - 4][:, :, c * 512:(c + 1) * 512],
                                start=(p == 4), stop=(p == 7),
                                perf_mode=DRMODE,
                            )

                # --- aux tail: xwi last so it covers the comb eviction ---
                for k in range(8):
                    if first:
                        load_w_until(28 + k + 3)
                    lhs = xb_t[:, k, :]
                    for c in range(2):
                        nc.tensor.matmul(
                            pa[:, c * 512:(c + 1) * 512],
                            lhs,
                            wi_t[k][:, c * 512:(c + 1) * 512],
                            start=(k == 0), stop=(k == 7),
                        )

                if first:
                    nc.sync.dma_start(bib_t[:], bib[:])
                    nc.sync.dma_start(bhb_t[:], bhb[:])
                    if not identity_ln:
                        nc.sync.dma_start(lnw_t[:], lnw[:])
                        nc.sync.dma_start(lnb_t[:], lnb[:])
                    # var is at WSCALE^2; fold into eps
                    nc.vector.memset(eps_t[:], LN_EPS * WSCALE * WSCALE)

                # --- epilogue ---
                if mm_only:
                    junk = sp.tile([128, 3], F32, tag="junk")
                    nc.vector.tensor_copy(junk[:, 0:1], pc[:, 0:1])
                    nc.vector.tensor_copy(junk[:, 1:2], pa[:, 0:1])
                    nc.vector.tensor_copy(junk[:, 2:3], pb[:, 0:1])
                    if bt_r == n_bt * reps - 1:
                        ot0 = ep.tile([128, O], F32, tag="ot")
                        nc.vector.tensor_copy(ot0[:], pa[:])
                        nc.sync.dma_start(
                            out[bt * 128:(bt + 1) * 128, :], ot0[:]
                        )
                    continue
                # evict comb (64x scale) to SBUF bf16 via ACT: frees the
                # pc banks ~2.4us after the comb phase, under the aux
                # phase of this tile -- the next tile's comb matmuls
                # then never stall on the epilogue reads.
                cS = sp.tile([128, 2048], BF, tag="cS")
                for i in range(4):
                    nc.scalar.activation(
                        cS[:, i * 512:(i + 1) * 512],
                        pc[:, i * 512:(i + 1) * 512],
                        AF.Copy,
                    )
                stats = sp.tile([128, 24], F32, tag="stats")
                for i in range(4):
                    nc.vector.bn_stats(
                        stats[:, i * 6:(i + 1) * 6],
                        cS[:, i * 512:(i + 1) * 512],
                    )
                mv = sp.tile([128, 2], F32, tag="mv")
                nc.vector.bn_aggr(mv[:], stats[:])
                std = sp.tile([128, 1], F32, tag="std")
                nc.scalar.activation(std[:], mv[:, 1:2], AF.Sqrt, bias=eps_t[:])
                rstd = sp.tile([128, 1], F32, tag="rstd")
                nc.vector.reciprocal(rstd[:], std[:])
                nmr = sp.tile([128, 1], F32, tag="nmr")
                # -mu * rstd
                nc.vector.scalar_tensor_tensor(
                    nmr[:], mv[:, 0:1], -1.0, rstd[:],
                    op0=ALU.mult, op1=ALU.mult,
                )

                gi = sp.tile([128, O], BF, tag="gi")
                gf = sp.tile([128, O], BF, tag="gf")
                if identity_ln:
                    # g = sigmoid(rstd*comb - mu*rstd); ig chunks first so
                    # the vt chain unblocks before the fg chunks finish
                    for i in range(4):
                        dst = gi if i < 2 else gf
                        nc.scalar.activation(
                            dst[:, (i % 2) * 512:(i % 2 + 1) * 512],
                            cS[:, i * 512:(i + 1) * 512],
                            AF.Sigmoid, bias=nmr[:], scale=rstd[:],
                        )
                else:
                    t1 = sp.tile([128, 2048], F32, tag="t1")
                    # (comb - mu) * lnw, then * rstd + lnb
                    nc.vector.scalar_tensor_tensor(
                        t1[:], cS[:], mv[:, 0:1], lnw_t[:],
                        op0=ALU.subtract, op1=ALU.mult,
                    )
                    nc.vector.scalar_tensor_tensor(
                        t1[:], t1[:], rstd[:], lnb_t[:],
                        op0=ALU.mult, op1=ALU.add,
                    )
                    nc.scalar.activation(gi[:], t1[:, 0:O], AF.Sigmoid)
                    nc.scalar.activation(gf[:], t1[:, O:2 * O], AF.Sigmoid)

                # gate-independent evictions first: the DVE stream is
                # in-order, so these fill the bubble while the sigmoid
                # (ACT) is still producing gi, and free pa/pb early.
                # v = (hwh64 / 64 + bh) ; evicts pb
                vt = ep.tile([128, O], F32, tag="vt")
                nc.vector.scalar_tensor_tensor(
                    vt[:], pb[:], 1.0 / WSCALE, bhb_t[:],
                    op0=ALU.mult, op1=ALU.add,
                )
                # c = xwi + bi ; evicts pa
                ct = ep.tile([128, O], F32, tag="ct")
                nc.vector.scalar_tensor_tensor(
                    ct[:], pa[:], 0.0, bib_t[:], op0=ALU.add, op1=ALU.add
                )
                nc.vector.tensor_mul(vt[:], vt[:], gi[:])
                nc.vector.tensor_add(vt[:], vt[:], ct[:])
                ht = ep.tile([128, O], BF, tag="ht")
                nc.scalar.activation(ht[:], vt[:], AF.Tanh)

                # out = hidden + fg * (state - hidden)
                dt = ep.tile([128, O], BF, tag="dt")
                nc.vector.tensor_sub(dt[:], st_t[:], ht[:])
                nc.vector.tensor_mul(dt[:], dt[:], gf[:])
                ot = ep.tile([128, O], F32, tag="ot")
                nc.vector.tensor_add(ot[:], ht[:], dt[:])
                nc.sync.dma_start(out[bt * 128:(bt + 1) * 128, :], ot[:])

    _split_multi_waits(nc)
    nc.finalize()
    return nc


_NC_CACHE = {}


def _get_nc(n_bt=N_BT, identity_ln=True):
    key = (n_bt, identity_ln)
    if key not in _NC_CACHE:
        _NC_CACHE[key] = build_program(n_bt, identity_ln=identity_ln)
    return _NC_CACHE[key]


def _prep_inputs(inpute, state, Wt, Wi, bi, Wh, bh, ln_w, ln_b):
    inpute = np.asarray(inpute, np.float32)
    state = np.asarray(state, np.float32)
    Wt = np.asarray(Wt, np.float32)
    Wi = np.asarray(Wi, np.float32)
    Wh = np.asarray(Wh, np.float32)

    identity_ln = bool(
        np.all(np.asarray(ln_w) == 1.0) and np.all(np.asarray(ln_b) == 0.0)
    )

    WtT = Wt.T  # [2048 (in: x then h), 2048 (out: ig then fg)]
    # ig fp8 weights at 64x, packed [pair*128+row, two, col]
    wig8 = (WSCALE * WtT[:, :O]).astype(E4M3).reshape(8, 2, 128, O)
    wig8 = np.ascontiguousarray(wig8.transpose(0, 2, 1, 3)).reshape(
        1024, 2, O
    )
    # fg bf16 weights at 64x (exact scaling)
    wfgb = (WSCALE * WtT[:, O:]).astype(BF16)
    # Wi bf16 true scale
    wib = np.ascontiguousarray(Wi.T).astype(BF16)
    # Wh fp8 at 64x, packed like wig
    wh8 = (WSCALE * Wh.T).astype(E4M3).reshape(4, 2, 128, O)
    wh8 = np.ascontiguousarray(wh8.transpose(0, 2, 1, 3)).reshape(512, 2, O)

    bib_b = np.broadcast_to(
        np.asarray(bi, np.float32).reshape(1, O), (128, O)
    ).astype(BF16)
    bhb_b = np.broadcast_to(
        np.asarray(bh, np.float32).reshape(1, O), (128, O)
    ).astype(BF16)
    lnw_b = np.broadcast_to(
        np.asarray(ln_w, np.float32).reshape(1, 2048), (128, 2048)
    ).astype(BF16)
    lnb_b = np.broadcast_to(
        np.asarray(ln_b, np.float32).reshape(1, 2048), (128, 2048)
    ).astype(BF16)

    in_maps = []
    for c in range(N_CORES):
        x_c = inpute[c * BL:(c + 1) * BL]
        h_c = state[c * BL:(c + 1) * BL]
        xh_t = np.empty((2048, BL), np.float32)
        xh_t[:1024] = x_c.T
        xh_t[1024:] = h_c.T
        m = {
            "x8": np.ascontiguousarray(xh_t.astype(E4M3)),
            "xb": np.ascontiguousarray(xh_t.astype(BF16)),
            "st": np.ascontiguousarray(h_c.astype(BF16)),
            "wig": wig8,
            "wfg": wfgb,
            "wi": wib,
            "wh": wh8,
            "bib": bib_b,
            "bhb": bhb_b,
        }
        if not identity_ln:
            m["lnw"] = lnw_b
            m["lnb"] = lnb_b
        in_maps.append(m)
    return in_maps, identity_ln


def run(inputs, trace=False, **trace_kwargs):
    in_maps, identity_ln = _prep_inputs(**inputs)
    nc = _get_nc(identity_ln=identity_ln)
    res = run_bass_kernel_spmd(
        nc, in_maps, list(range(N_CORES)), trace=trace, **trace_kwargs
    )
    out = np.concatenate([res.results[c]["out"] for c in range(N_CORES)], axis=0)
    return out, res


def kernel(**inputs):
    out, _ = run(inputs)
    return out
